# revision 15
# baseline (speedup 1.0000x reference)
"""DGCNLayer (layer%2==0 branch) on 8 Trainium2 NeuronCores via Bass.

Math (per reference, with uv_vals == 1 and using linearity to pull the
dense GEMM past the segment-sum):
  User_n = leaky_relu(segsum_{rows}(vfea[cols]) @ W1 + b1, 0.1)
  Item_n = leaky_relu(segsum_{cols}(ufea[rows]) @ W2 + b2, 0.1)
  User_h = relu(concat([ufea, User_n]) @ Wu + bu)
  Item_h = relu(concat([vfea, Item_n]) @ Wi + bi)
  return stack([User_h, User_n, ufea, Item_h, Item_n, vfea])

Distribution: destinations sharded 12500/core. Per-row descriptor
gathers are the hard bottleneck on TRN2 (GPSIMD indirect DMA ~1.1us per
128 rows; dma_gather ucode ~27ns/row), so the host performs the edge
gather as part of input sharding: per (core, direction) it emits a
destination-major message stream msgs[p, (t,r), f] = fea[src of the
r-th edge of dst p in tile t] (zero rows pad short degrees), with the
12500 destinations permuted by descending degree so round counts per
128-dst tile track the tile's max degree (sum_t R[t] ~ E/128). The
device then streams msgs sequentially (large static HWDGE DMAs, full
bandwidth - the memory-roofline realization of this memory-regime
problem) and aggregates on TensorE: psA[f,d] += mt_r[d,f] via an
accumulating matmul with a constant bf16 identity as the moving
operand (a transpose-accumulate; no one-hot build, Vector idle, Pool
idle). The dense tail per 256-dst tile pair: ScalarE copies psum->SBUF
as f32r, one f32r matmul applies W1, ScalarE applies LeakyRelu(+b1)
straight from PSUM (Prelu table, alpha=0.1), the union accumulates a
bf16 (features) and an f32r (User_n) matmul into one psum, and ScalarE
finishes with Relu(+bu). Outputs return in permuted dst order; the
host inverts the permutation while assembling the full stack.
"""
import sys
sys.path.insert(0, "/opt/trn_rl_repo")
import numpy as np

from concourse import bass, bacc, mybir
from concourse import bass_utils
from concourse.tile import TileContext

F32 = mybir.dt.float32
F32R = mybir.dt.float32r
BF16 = mybir.dt.bfloat16

NCORES = 8
N_NODES = 100000
SH = N_NODES // NCORES      # 12500 destinations per core
D = 128
TW = 128                    # dst tile width (PE contraction width)
AGG_ROWS = 12544            # 12500 padded to x128
NT = AGG_ROWS // TW         # 98 dst tiles
G = 14                      # dst tiles per stream group
NGRP = NT // G              # 7 groups
ALPHA = 0.1


def _prep_direction(dst_all, src_all, fea_bf16_pad):
    """Host-side edge gather into destination-major rounds.

    Returns (R[t] rounds per tile shared across cores, per-core
    (msgs [128, totch*128] bf16, perm [12500]), colbase, totch)."""
    cores = []
    for c in range(NCORES):
        m = (dst_all >= c * SH) & (dst_all < (c + 1) * SH)
        d = (dst_all[m] - c * SH).astype(np.int64)
        s = src_all[m].astype(np.int64)
        deg = np.bincount(d, minlength=SH)
        perm = np.argsort(-deg, kind="stable")
        rank = np.empty(SH, np.int64)
        rank[perm] = np.arange(SH)
        q = rank[d]
        o = np.argsort(q, kind="stable")
        qs, ss = q[o], s[o]
        start = np.searchsorted(qs, np.arange(SH))
        r = np.arange(len(qs)) - start[qs]
        cores.append((qs, ss, r, perm, deg[perm]))

    R = [max(1, max(int(cc[4][t * TW]) for cc in cores)) for t in range(NT - 1)]
    R.append(max(1, max(int(cc[4][(NT - 1) * TW]) for cc in cores
                        if (NT - 1) * TW < SH)))
    colbase = np.zeros(NT + 1, np.int64)
    colbase[1:] = np.cumsum(R)
    totch = int(colbase[NT])

    per_core = []
    for (qs, ss, r, perm, _) in cores:
        grid = np.full((128, totch), N_NODES, np.int64)
        t_e = qs >> 7
        p_e = qs & 127
        grid[p_e, colbase[t_e] + r] = ss
        msgs = fea_bf16_pad[grid]            # [128, totch, 128] bf16
        per_core.append((np.ascontiguousarray(
            msgs.reshape(128, totch * D)), perm))
    return R, per_core, colbase, totch


def _build(nc: bass.Bass, R_u, R_i, cb_u, cb_i, totch_u, totch_i, cgmax):
    hmax = [0, 0]
    for cb in (cb_u, cb_i):
        for g in range(NGRP):
            hmax[0] = max(hmax[0], int(cb[g * G + 8]) - int(cb[g * G]))
            hmax[1] = max(hmax[1], int(cb[(g + 1) * G]) - int(cb[g * G + 8]))
    msgs_u = nc.dram_tensor("msgs_u", [128, totch_u * D], BF16,
                            kind="ExternalInput")
    msgs_i = nc.dram_tensor("msgs_i", [128, totch_i * D], BF16,
                            kind="ExternalInput")
    ufeaT = nc.dram_tensor("ufeaT", [128, AGG_ROWS], BF16, kind="ExternalInput")
    vfeaT = nc.dram_tensor("vfeaT", [128, AGG_ROWS], BF16, kind="ExternalInput")
    ident = nc.dram_tensor("ident", [128, 128], BF16, kind="ExternalInput")
    wn = {}
    for w in ("W1", "W2", "Wu_b", "Wi_b"):
        wn[w] = nc.dram_tensor(w, [128, 128], F32, kind="ExternalInput")
    for w in ("Wu_t", "Wi_t"):
        wn[w] = nc.dram_tensor(w, [128, 128], BF16, kind="ExternalInput")
    for b in ("b1", "b2", "bu", "bi"):
        wn[b] = nc.dram_tensor(b, [128, 1], F32, kind="ExternalInput")

    unT = nc.dram_tensor("unT", [128, AGG_ROWS], BF16, kind="ExternalOutput")
    uhT = nc.dram_tensor("uhT", [128, AGG_ROWS], BF16, kind="ExternalOutput")
    inT = nc.dram_tensor("inT", [128, AGG_ROWS], BF16, kind="ExternalOutput")
    ihT = nc.dram_tensor("ihT", [128, AGG_ROWS], BF16, kind="ExternalOutput")

    with TileContext(nc) as tc:
        with (
            tc.tile_pool(name="wts", bufs=1) as wtsp,
            tc.tile_pool(name="slab", bufs=2) as slabp,
            tc.tile_pool(name="agt", bufs=4) as agtp,
            tc.tile_pool(name="cmp", bufs=4) as cmpp,
            tc.tile_pool(name="ftg", bufs=2) as ftgp,
            tc.tile_pool(name="outg", bufs=2) as outgp,
            tc.tile_pool(name="agg", bufs=4, space="PSUM") as aggp,
            tc.tile_pool(name="mmp", bufs=2, space="PSUM") as mmpp,
            tc.tile_pool(name="php", bufs=2, space="PSUM") as phpp,
        ):
            w = {}
            for name in ("W1", "W2", "Wu_b", "Wi_b"):
                w[name] = wtsp.tile([128, 128], F32R, tag=name,
                                    name=f"w_{name}")
                nc.sync.dma_start(w[name][:], wn[name][:].bitcast(F32R))
            for name in ("Wu_t", "Wi_t"):
                w[name] = wtsp.tile([128, 128], BF16, tag=name,
                                    name=f"w_{name}")
                nc.sync.dma_start(w[name][:], wn[name][:])
            for name in ("b1", "b2", "bu", "bi"):
                w[name] = wtsp.tile([128, 1], F32, tag=name, name=f"w_{name}")
                nc.sync.dma_start(w[name][:], wn[name][:])
            t_id = wtsp.tile([128, 128], BF16, tag="ident")
            nc.sync.dma_start(t_id[:], ident[:])

            def emit_group(key, msgs, R, cb, feaT, W1n, b1n, Wtn, Wbn,
                           btn, nT_out, hT_out, gi):
                grp = list(range(gi * G, (gi + 1) * G))
                halves = (grp[:8], grp[8:])
                with nc.named_scope(f"dir_{key}"):
                    slabs = {}
                    for hi, hgrp in enumerate(halves):
                        h0 = int(cb[hgrp[0]])
                        hc = int(cb[hgrp[-1] + 1]) - h0
                        tg = "slabA" if hi == 0 else "slabB"
                        sl = slabp.tile([128, hmax[hi] * D], BF16, tag=tg)
                        nc.sync.dma_start(
                            sl[:, :hc * D],
                            msgs[:, h0 * D:(h0 + hc) * D])
                        for t in hgrp:
                            slabs[t] = (sl, h0)
                    ftg = ftgp.tile([128, G * TW], BF16, tag="ftg")
                    nc.scalar.dma_start(
                        ftg[:], feaT[:, grp[0] * TW:(grp[-1] + 1) * TW])
                    nTg = outgp.tile([128, G * TW], F32R, tag="nTg")
                    hTg = outgp.tile([128, G * TW], BF16, tag="hTg")

                    for pi in range(0, G, 2):
                        pair = grp[pi:pi + 2]
                        aggT = agtp.tile([128, 2 * TW], F32R, tag="aggT")
                        for h, t in enumerate(pair):
                            slab, h0 = slabs[t]
                            psA = aggp.tile([128, TW], F32, tag="psA")
                            for r in range(R[t]):
                                col = int(cb[t]) - h0 + r
                                nc.tensor.matmul(
                                    psA[:],
                                    slab[:, col * D:(col + 1) * D],
                                    t_id[:],
                                    start=(r == 0), stop=(r == R[t] - 1))
                            nc.vector.tensor_copy(
                                aggT[:, h * TW:(h + 1) * TW], psA[:])

                        pn = mmpp.tile([128, 2 * TW], F32, tag="pn")
                        nc.tensor.matmul(pn[:], w[W1n][:], aggT[:],
                                         start=True, stop=True)
                        nT = nTg[:, pi * TW:(pi + 2) * TW]
                        nc.scalar.activation(
                            nT, pn[:],
                            mybir.ActivationFunctionType.Prelu,
                            bias=w[b1n][:], scale=1.0, alpha=ALPHA)

                        ph = phpp.tile([128, 2 * TW], F32, tag="ph")
                        nc.tensor.matmul(
                            ph[:], w[Wtn][:],
                            ftg[:, pi * TW:(pi + 2) * TW],
                            start=True, stop=False)
                        nc.tensor.matmul(ph[:], w[Wbn][:], nT,
                                         start=False, stop=True)
                        nc.scalar.activation(
                            hTg[:, pi * TW:(pi + 2) * TW], ph[:],
                            mybir.ActivationFunctionType.Relu,
                            bias=w[btn][:], scale=1.0)

                    jg = grp[0] * TW
                    nTb = outgp.tile([128, G * TW], BF16, tag="nTb")
                    nc.vector.tensor_copy(nTb[:], nTg[:].bitcast(F32))
                    nc.scalar.dma_start(nT_out[:, jg:jg + G * TW], nTb[:])
                    nc.sync.dma_start(hT_out[:, jg:jg + G * TW], hTg[:])

            for gi in range(NGRP):
                emit_group("u", msgs_u, R_u, cb_u, ufeaT, "W1", "b1",
                           "Wu_t", "Wu_b", "bu", unT, uhT, gi)
                emit_group("i", msgs_i, R_i, cb_i, vfeaT, "W2", "b2",
                           "Wi_t", "Wi_b", "bi", inT, ihT, gi)
    return nc


def kernel(ufea, vfea, uv_rows, uv_cols, uv_vals,
           W1, b1, W2, b2, Wu, bu, Wi, bi) -> np.ndarray:
    import ml_dtypes
    ufea = np.ascontiguousarray(np.asarray(ufea, np.float32))
    vfea = np.ascontiguousarray(np.asarray(vfea, np.float32))
    uv_rows = np.asarray(uv_rows, np.int64)
    uv_cols = np.asarray(uv_cols, np.int64)

    # uv_vals is all-ones per the problem spec; the message rows would be
    # scaled here otherwise.
    vpad = np.concatenate(
        [vfea, np.zeros((1, D), np.float32)]).astype(ml_dtypes.bfloat16)
    upad = np.concatenate(
        [ufea, np.zeros((1, D), np.float32)]).astype(ml_dtypes.bfloat16)
    R_u, pc_u, cb_u, totch_u = _prep_direction(uv_rows, uv_cols, vpad)
    R_i, pc_i, cb_i, totch_i = _prep_direction(uv_cols, uv_rows, upad)

    def cgroups(cb):
        return max(int(cb[(g + 1) * G]) - int(cb[g * G]) for g in range(NGRP))
    cgmax = max(cgroups(cb_u), cgroups(cb_i))

    nc = bacc.Bacc("TRN2", target_bir_lowering=False, debug=False,
                   dynamic_dma_scratch_size=2**14)
    _build(nc, R_u, R_i, cb_u, cb_i, totch_u, totch_i, cgmax)
    nc.compile()

    Wu = np.asarray(Wu, np.float32)
    Wi = np.asarray(Wi, np.float32)
    common = {
        "ident": np.eye(128, dtype=np.float32).astype(ml_dtypes.bfloat16),
        "W1": np.asarray(W1, np.float32), "W2": np.asarray(W2, np.float32),
        "Wu_t": np.ascontiguousarray(Wu[:128]).astype(ml_dtypes.bfloat16),
        "Wu_b": np.ascontiguousarray(Wu[128:]),
        "Wi_t": np.ascontiguousarray(Wi[:128]).astype(ml_dtypes.bfloat16),
        "Wi_b": np.ascontiguousarray(Wi[128:]),
        "b1": np.asarray(b1, np.float32).reshape(128, 1),
        "b2": np.asarray(b2, np.float32).reshape(128, 1),
        "bu": np.asarray(bu, np.float32).reshape(128, 1),
        "bi": np.asarray(bi, np.float32).reshape(128, 1),
    }
    in_maps = []
    for c in range(NCORES):
        m = dict(common)
        m["msgs_u"], perm_u = pc_u[c]
        m["msgs_i"], perm_i = pc_i[c]
        fu = np.zeros((128, AGG_ROWS), ml_dtypes.bfloat16)
        fu[:, :SH] = ufea[c * SH + perm_u].T.astype(ml_dtypes.bfloat16)
        fv = np.zeros((128, AGG_ROWS), ml_dtypes.bfloat16)
        fv[:, :SH] = vfea[c * SH + perm_i].T.astype(ml_dtypes.bfloat16)
        m["ufeaT"], m["vfeaT"] = fu, fv
        in_maps.append(m)

    res = bass_utils.run_bass_kernel_spmd(nc, in_maps, list(range(NCORES)),
                                          trace=False)

    out = np.empty((6, N_NODES, D), np.float32)
    for c in range(NCORES):
        r = res.results[c]
        perm_u, perm_i = pc_u[c][1], pc_i[c][1]
        su = slice(c * SH, (c + 1) * SH)
        o0 = out[0][su]; o1 = out[1][su]; o3 = out[3][su]; o4 = out[4][su]
        o0[perm_u] = r["uhT"][:, :SH].T.astype(np.float32)
        o1[perm_u] = r["unT"][:, :SH].T.astype(np.float32)
        o3[perm_i] = r["ihT"][:, :SH].T.astype(np.float32)
        o4[perm_i] = r["inT"][:, :SH].T.astype(np.float32)
    out[2] = ufea
    out[5] = vfea
    return out


# revision 16
# speedup vs baseline: 1.0386x; 1.0386x over previous
"""DGCNLayer (layer%2==0 branch) on 8 Trainium2 NeuronCores via Bass.

Math (per reference, with uv_vals == 1 and using linearity to pull the
dense GEMM past the segment-sum):
  User_n = leaky_relu(segsum_{rows}(vfea[cols]) @ W1 + b1, 0.1)
  Item_n = leaky_relu(segsum_{cols}(ufea[rows]) @ W2 + b2, 0.1)
  User_h = relu(concat([ufea, User_n]) @ Wu + bu)
  Item_h = relu(concat([vfea, Item_n]) @ Wi + bi)
  return stack([User_h, User_n, ufea, Item_h, Item_n, vfea])

Distribution: destinations sharded 12500/core. Per-row descriptor
gathers are the hard bottleneck on TRN2 (GPSIMD indirect DMA ~1.1us per
128 rows; dma_gather ucode ~27ns/row), so the host performs the edge
gather as part of input sharding: per (core, direction) it emits a
destination-major message stream msgs[p, (t,r), f] = fea[src of the
r-th edge of dst p in tile t] (zero rows pad short degrees), with the
12500 destinations permuted by descending degree so round counts per
128-dst tile track the tile's max degree (sum_t R[t] ~ E/128). The
device then streams msgs sequentially (large static HWDGE DMAs, full
bandwidth - the memory-roofline realization of this memory-regime
problem) and aggregates on TensorE: psA[f,d] += mt_r[d,f] via an
accumulating matmul with a constant bf16 identity as the moving
operand (a transpose-accumulate; no one-hot build, Vector idle, Pool
idle). The dense tail per 256-dst tile pair: ScalarE copies psum->SBUF
as f32r, one f32r matmul applies W1, ScalarE applies LeakyRelu(+b1)
straight from PSUM (Prelu table, alpha=0.1), the union accumulates a
bf16 (features) and an f32r (User_n) matmul into one psum, and ScalarE
finishes with Relu(+bu). Outputs return in permuted dst order; the
host inverts the permutation while assembling the full stack.
"""
import sys
sys.path.insert(0, "/opt/trn_rl_repo")
import numpy as np

from concourse import bass, bacc, mybir
from concourse import bass_utils
from concourse.tile import TileContext

F32 = mybir.dt.float32
F32R = mybir.dt.float32r
BF16 = mybir.dt.bfloat16

NCORES = 8
N_NODES = 100000
SH = N_NODES // NCORES      # 12500 destinations per core
D = 128
TW = 128                    # dst tile width (PE contraction width)
AGG_ROWS = 12544            # 12500 padded to x128
NT = AGG_ROWS // TW         # 98 dst tiles
G = 14                      # dst tiles per stream group
NGRP = NT // G              # 7 groups
ALPHA = 0.1


def _prep_direction(dst_all, src_all, fea_bf16_pad):
    """Host-side edge gather into destination-major rounds.

    Returns (R[t] rounds per tile shared across cores, per-core
    (msgs [128, totch*128] bf16, perm [12500]), colbase, totch)."""
    cores = []
    for c in range(NCORES):
        m = (dst_all >= c * SH) & (dst_all < (c + 1) * SH)
        d = (dst_all[m] - c * SH).astype(np.int64)
        s = src_all[m].astype(np.int64)
        deg = np.bincount(d, minlength=SH)
        perm = np.argsort(-deg, kind="stable")
        rank = np.empty(SH, np.int64)
        rank[perm] = np.arange(SH)
        q = rank[d]
        o = np.argsort(q, kind="stable")
        qs, ss = q[o], s[o]
        start = np.searchsorted(qs, np.arange(SH))
        r = np.arange(len(qs)) - start[qs]
        cores.append((qs, ss, r, perm, deg[perm]))

    R = [max(1, max(int(cc[4][t * TW]) for cc in cores)) for t in range(NT - 1)]
    R.append(max(1, max(int(cc[4][(NT - 1) * TW]) for cc in cores
                        if (NT - 1) * TW < SH)))
    colbase = np.zeros(NT + 1, np.int64)
    colbase[1:] = np.cumsum(R)
    totch = int(colbase[NT])

    per_core = []
    for (qs, ss, r, perm, _) in cores:
        grid = np.full((128, totch), N_NODES, np.int64)
        t_e = qs >> 7
        p_e = qs & 127
        grid[p_e, colbase[t_e] + r] = ss
        msgs = fea_bf16_pad[grid]            # [128, totch, 128] bf16
        per_core.append((np.ascontiguousarray(
            msgs.reshape(128, totch * D)), perm))
    return R, per_core, colbase, totch


def _build(nc: bass.Bass, R_u, R_i, cb_u, cb_i, totch_u, totch_i, cgmax):
    hmax = [0, 0]
    for cb in (cb_u, cb_i):
        for g in range(NGRP):
            hmax[0] = max(hmax[0], int(cb[g * G + 8]) - int(cb[g * G]))
            hmax[1] = max(hmax[1], int(cb[(g + 1) * G]) - int(cb[g * G + 8]))
    msgs_u = nc.dram_tensor("msgs_u", [128, totch_u * D], BF16,
                            kind="ExternalInput")
    msgs_i = nc.dram_tensor("msgs_i", [128, totch_i * D], BF16,
                            kind="ExternalInput")
    ufeaT = nc.dram_tensor("ufeaT", [128, AGG_ROWS], BF16, kind="ExternalInput")
    vfeaT = nc.dram_tensor("vfeaT", [128, AGG_ROWS], BF16, kind="ExternalInput")
    ident = nc.dram_tensor("ident", [128, 128], BF16, kind="ExternalInput")
    wn = {}
    for w in ("W1", "W2", "Wu_b", "Wi_b"):
        wn[w] = nc.dram_tensor(w, [128, 128], F32, kind="ExternalInput")
    for w in ("Wu_t", "Wi_t"):
        wn[w] = nc.dram_tensor(w, [128, 128], BF16, kind="ExternalInput")
    for b in ("b1", "b2", "bu", "bi"):
        wn[b] = nc.dram_tensor(b, [128, 1], F32, kind="ExternalInput")

    unT = nc.dram_tensor("unT", [128, AGG_ROWS], BF16, kind="ExternalOutput")
    uhT = nc.dram_tensor("uhT", [128, AGG_ROWS], BF16, kind="ExternalOutput")
    inT = nc.dram_tensor("inT", [128, AGG_ROWS], BF16, kind="ExternalOutput")
    ihT = nc.dram_tensor("ihT", [128, AGG_ROWS], BF16, kind="ExternalOutput")

    with TileContext(nc) as tc:
        with (
            tc.tile_pool(name="wts", bufs=1) as wtsp,
            tc.tile_pool(name="slab", bufs=2) as slabp,
            tc.tile_pool(name="agt", bufs=4) as agtp,
            tc.tile_pool(name="cmp", bufs=4) as cmpp,
            tc.tile_pool(name="ftg", bufs=2) as ftgp,
            tc.tile_pool(name="outg", bufs=2) as outgp,
            tc.tile_pool(name="agg", bufs=4, space="PSUM") as aggp,
            tc.tile_pool(name="mmp", bufs=2, space="PSUM") as mmpp,
            tc.tile_pool(name="php", bufs=2, space="PSUM") as phpp,
        ):
            w = {}
            for name in ("W1", "W2", "Wu_b", "Wi_b"):
                w[name] = wtsp.tile([128, 128], F32R, tag=name,
                                    name=f"w_{name}")
                nc.sync.dma_start(w[name][:], wn[name][:].bitcast(F32R))
            for name in ("Wu_t", "Wi_t"):
                w[name] = wtsp.tile([128, 128], BF16, tag=name,
                                    name=f"w_{name}")
                nc.sync.dma_start(w[name][:], wn[name][:])
            for name in ("b1", "b2", "bu", "bi"):
                w[name] = wtsp.tile([128, 1], F32, tag=name, name=f"w_{name}")
                nc.sync.dma_start(w[name][:], wn[name][:])
            t_id = wtsp.tile([128, 128], BF16, tag="ident")
            nc.sync.dma_start(t_id[:], ident[:])

            def emit_group(key, msgs, R, cb, feaT, W1n, b1n, Wtn, Wbn,
                           btn, nT_out, hT_out, gi):
                grp = list(range(gi * G, (gi + 1) * G))
                halves = (grp[:8], grp[8:])
                with nc.named_scope(f"dir_{key}"):
                    ftg = ftgp.tile([128, G * TW], BF16, tag="ftg")
                    nc.scalar.dma_start(
                        ftg[:], feaT[:, grp[0] * TW:(grp[-1] + 1) * TW])
                    slabs = {}
                    for hi, hgrp in enumerate(halves):
                        h0 = int(cb[hgrp[0]])
                        hc = int(cb[hgrp[-1] + 1]) - h0
                        tg = "slabA" if hi == 0 else "slabB"
                        sl = slabp.tile([128, hmax[hi] * D], BF16, tag=tg)
                        nc.sync.dma_start(
                            sl[:, :hc * D],
                            msgs[:, h0 * D:(h0 + hc) * D])
                        for t in hgrp:
                            slabs[t] = (sl, h0)
                    nTg = outgp.tile([128, G * TW], F32R, tag="nTg")
                    hTg = outgp.tile([128, G * TW], BF16, tag="hTg")

                    def tail(pi, aggT):
                        pn = mmpp.tile([128, 2 * TW], F32, tag="pn")
                        nc.tensor.matmul(pn[:], w[W1n][:], aggT[:],
                                         start=True, stop=True)
                        nT = nTg[:, pi * TW:(pi + 2) * TW]
                        nc.scalar.activation(
                            nT, pn[:],
                            mybir.ActivationFunctionType.Prelu,
                            bias=w[b1n][:], scale=1.0, alpha=ALPHA)
                        ph = phpp.tile([128, 2 * TW], F32, tag="ph")
                        nc.tensor.matmul(
                            ph[:], w[Wtn][:],
                            ftg[:, pi * TW:(pi + 2) * TW],
                            start=True, stop=False)
                        nc.tensor.matmul(ph[:], w[Wbn][:], nT,
                                         start=False, stop=True)
                        nc.scalar.activation(
                            hTg[:, pi * TW:(pi + 2) * TW], ph[:],
                            mybir.ActivationFunctionType.Relu,
                            bias=w[btn][:], scale=1.0)

                    # software-pipeline: emit pair k's aggregation, then
                    # pair k-1's dense tail, so the in-order TensorE queue
                    # never head-of-line blocks on tail inputs
                    pend = None
                    for pi in range(0, G, 2):
                        pair = grp[pi:pi + 2]
                        aggT = agtp.tile([128, 2 * TW], F32R, tag="aggT")
                        for h, t in enumerate(pair):
                            slab, h0 = slabs[t]
                            psA = aggp.tile([128, TW], F32, tag="psA")
                            for r in range(R[t]):
                                col = int(cb[t]) - h0 + r
                                nc.tensor.matmul(
                                    psA[:],
                                    slab[:, col * D:(col + 1) * D],
                                    t_id[:],
                                    start=(r == 0), stop=(r == R[t] - 1))
                            nc.vector.tensor_copy(
                                aggT[:, h * TW:(h + 1) * TW], psA[:])
                        if pend is not None:
                            tail(*pend)
                        pend = (pi, aggT)
                    tail(*pend)

                    jg = grp[0] * TW
                    nTb = outgp.tile([128, G * TW], BF16, tag="nTb")
                    nc.vector.tensor_copy(nTb[:], nTg[:].bitcast(F32))
                    nc.scalar.dma_start(nT_out[:, jg:jg + G * TW], nTb[:])
                    nc.sync.dma_start(hT_out[:, jg:jg + G * TW], hTg[:])

            for gi in range(NGRP):
                emit_group("u", msgs_u, R_u, cb_u, ufeaT, "W1", "b1",
                           "Wu_t", "Wu_b", "bu", unT, uhT, gi)
                emit_group("i", msgs_i, R_i, cb_i, vfeaT, "W2", "b2",
                           "Wi_t", "Wi_b", "bi", inT, ihT, gi)
    return nc


def kernel(ufea, vfea, uv_rows, uv_cols, uv_vals,
           W1, b1, W2, b2, Wu, bu, Wi, bi) -> np.ndarray:
    import ml_dtypes
    ufea = np.ascontiguousarray(np.asarray(ufea, np.float32))
    vfea = np.ascontiguousarray(np.asarray(vfea, np.float32))
    uv_rows = np.asarray(uv_rows, np.int64)
    uv_cols = np.asarray(uv_cols, np.int64)

    # uv_vals is all-ones per the problem spec; the message rows would be
    # scaled here otherwise.
    vpad = np.concatenate(
        [vfea, np.zeros((1, D), np.float32)]).astype(ml_dtypes.bfloat16)
    upad = np.concatenate(
        [ufea, np.zeros((1, D), np.float32)]).astype(ml_dtypes.bfloat16)
    R_u, pc_u, cb_u, totch_u = _prep_direction(uv_rows, uv_cols, vpad)
    R_i, pc_i, cb_i, totch_i = _prep_direction(uv_cols, uv_rows, upad)

    def cgroups(cb):
        return max(int(cb[(g + 1) * G]) - int(cb[g * G]) for g in range(NGRP))
    cgmax = max(cgroups(cb_u), cgroups(cb_i))

    nc = bacc.Bacc("TRN2", target_bir_lowering=False, debug=False,
                   dynamic_dma_scratch_size=2**14)
    _build(nc, R_u, R_i, cb_u, cb_i, totch_u, totch_i, cgmax)
    nc.compile()

    Wu = np.asarray(Wu, np.float32)
    Wi = np.asarray(Wi, np.float32)
    common = {
        "ident": np.eye(128, dtype=np.float32).astype(ml_dtypes.bfloat16),
        "W1": np.asarray(W1, np.float32), "W2": np.asarray(W2, np.float32),
        "Wu_t": np.ascontiguousarray(Wu[:128]).astype(ml_dtypes.bfloat16),
        "Wu_b": np.ascontiguousarray(Wu[128:]),
        "Wi_t": np.ascontiguousarray(Wi[:128]).astype(ml_dtypes.bfloat16),
        "Wi_b": np.ascontiguousarray(Wi[128:]),
        "b1": np.asarray(b1, np.float32).reshape(128, 1),
        "b2": np.asarray(b2, np.float32).reshape(128, 1),
        "bu": np.asarray(bu, np.float32).reshape(128, 1),
        "bi": np.asarray(bi, np.float32).reshape(128, 1),
    }
    in_maps = []
    for c in range(NCORES):
        m = dict(common)
        m["msgs_u"], perm_u = pc_u[c]
        m["msgs_i"], perm_i = pc_i[c]
        fu = np.zeros((128, AGG_ROWS), ml_dtypes.bfloat16)
        fu[:, :SH] = ufea[c * SH + perm_u].T.astype(ml_dtypes.bfloat16)
        fv = np.zeros((128, AGG_ROWS), ml_dtypes.bfloat16)
        fv[:, :SH] = vfea[c * SH + perm_i].T.astype(ml_dtypes.bfloat16)
        m["ufeaT"], m["vfeaT"] = fu, fv
        in_maps.append(m)

    res = bass_utils.run_bass_kernel_spmd(nc, in_maps, list(range(NCORES)),
                                          trace=False)

    out = np.empty((6, N_NODES, D), np.float32)
    for c in range(NCORES):
        r = res.results[c]
        perm_u, perm_i = pc_u[c][1], pc_i[c][1]
        su = slice(c * SH, (c + 1) * SH)
        o0 = out[0][su]; o1 = out[1][su]; o3 = out[3][su]; o4 = out[4][su]
        o0[perm_u] = r["uhT"][:, :SH].T.astype(np.float32)
        o1[perm_u] = r["unT"][:, :SH].T.astype(np.float32)
        o3[perm_i] = r["ihT"][:, :SH].T.astype(np.float32)
        o4[perm_i] = r["inT"][:, :SH].T.astype(np.float32)
    out[2] = ufea
    out[5] = vfea
    return out
